# revision 1
# baseline (speedup 1.0000x reference)
"""Trainium2 Bass kernel for BEiT-style dot-product attention with relative
position bias (batch 8, seq 1025, dim 1024, 16 heads).

Strategy: data-parallel — one batch element per NeuronCore (8 cores).
Per core, everything runs in fp32r (12-bit-mantissa fp32) matmuls at full PE
rate, except the attention-weight path (exp output / V) which is bf16.

Layouts (per core):
  xT   [1025, 1152]  : x[b].T padded to seq 1152 (9 j-tiles), plus a ones row
                       (row 1024) that folds the qkv bias add into a K=1 matmul.
  q/k  computed transposed [chan, seq]; v computed natural [seq, chan], stored
       per-head with a ones column appended ([v_h | 1]) so the PV matmul
       (M=65) emits the softmax denominator as psum row 64 for free.
  scores S.T [j, i] per head via row-packed (2 heads share the PE array rows)
       fp32r QK matmuls + a bf16 identity matmul that adds 8*bias into psum.
  exp via ScalarE (scale=1/8) -> bf16 attn tiles; PV accumulates over the 9
       j-tiles; normalization by 1/denominator is broadcast across partitions
       with a K=2 selector matmul and applied by VectorE during psum drain.
  proj y = outT.T @ proj_w.T + proj_b (ones-row K=1 trick again).
"""

import os
import sys

for _p in (
    "/root/.axon_site",
    "/root/.axon_site/_ro/trn_rl_repo",
    "/root/.axon_site/_ro/pypackages",
    "/opt/trn_rl_repo",
    "/opt/pypackages",
):
    if os.path.isdir(_p) and _p not in sys.path:
        sys.path.append(_p)

import numpy as np
import ml_dtypes

import concourse.bass as bass
import concourse.bacc as bacc
import concourse.tile as tile
import concourse.mybir as mybir
from concourse.bass_utils import run_bass_kernel_spmd

F32 = mybir.dt.float32
F32R = mybir.dt.float32r
BF16 = mybir.dt.bfloat16
F16 = mybir.dt.float16
EXPFN = mybir.ActivationFunctionType.Exp

SEQ = 1025          # 32*32 grid + 1 cls token
SP = 1152           # padded seq (9 j-tiles of 128)
D = 1024
H = 16
NB = 8              # batch == cores
NJT = SP // 128     # 9
IB = [(0, 512), (512, 512)]                  # full i-blocks; i=1024 special
KB = [(0, 512), (512, 384), (896, 256)]      # k-production j-blocks
CBV = [(0, 384), (384, 384), (768, 256)]     # v-production channel blocks
FB = [(0, 512), (512, 512)]                  # proj output-channel blocks
NEG = -6.0e30                                # pad-key bias (*1/8 still -inf-ish)

_CACHE = {}


def _f32r_round(a):
    u = np.ascontiguousarray(a, dtype=np.float32).view(np.uint32)
    r = (u.astype(np.uint64) + 0x7FF + ((u >> 12) & 1)) & 0xFFFFF000
    return r.astype(np.uint32).view(np.float32)


def _build_module():
    nc = bacc.Bacc()
    xt_d = nc.dram_tensor("xt", [SEQ, SP], F32R, kind="ExternalInput")
    wqk_d = nc.dram_tensor("wqk", [SEQ, 2 * D], F32R, kind="ExternalInput")
    wv_d = nc.dram_tensor("wv", [SEQ, D], F32R, kind="ExternalInput")
    wp_d = nc.dram_tensor("wp", [SEQ, D], F32R, kind="ExternalInput")
    bias_d = nc.dram_tensor("biasT", [H, SP, SEQ], BF16, kind="ExternalInput")
    bias1_d = nc.dram_tensor("bias1", [H, SP, 2], BF16, kind="ExternalInput")
    iden_d = nc.dram_tensor("iden", [128, 128], BF16, kind="ExternalInput")
    y_d = nc.dram_tensor("y", [SEQ, D], F32, kind="ExternalOutput")

    with tile.TileContext(nc) as tc:
        with (
            tc.tile_pool(name="persist", bufs=1) as pp,
            tc.tile_pool(name="consts", bufs=1) as cp,
        ):
            qt = pp.tile([128, 8, SEQ], F32R, tag="qt")
            kt = pp.tile([128, 8, SP], F32R, tag="kt")
            va = pp.tile([128, NJT, H, 65], F16, tag="va")
            out1 = pp.tile([128, 8, 1], F32R, tag="out1")

            iden = cp.tile([128, 128], BF16, tag="iden")
            ones_col_f = cp.tile([128, NJT * H], F32, tag="onescolf")
            ones_row_f = cp.tile([1, 512], F32, tag="onesrowf")
            ones_row = cp.tile([1, 512], F32R, tag="onesrow")
            wql = cp.tile([1, 2 * D], F32R, tag="wql")
            wvl = cp.tile([1, D], F32R, tag="wvl")
            wpl = cp.tile([1, D], F32R, tag="wpl")
            nc.sync.dma_start(out=iden, in_=iden_d[:, :])
            nc.sync.dma_start(out=wql, in_=wqk_d[SEQ - 1 : SEQ, :])
            nc.sync.dma_start(out=wvl, in_=wv_d[SEQ - 1 : SEQ, :])
            nc.sync.dma_start(out=wpl, in_=wp_d[SEQ - 1 : SEQ, :])
            nc.vector.memset(ones_col_f, 1.0)
            nc.vector.memset(ones_row_f, 1.0)
            nc.vector.tensor_copy(ones_row, ones_row_f)
            # ones columns of v_aug ([128, 9, 16, 1] strided view)
            nc.vector.tensor_copy(
                va[:, :, :, 64:65],
                ones_col_f.rearrange("p (t h) -> p t h", t=NJT).unsqueeze(3),
            )

            # ---------------- Phase A: projections ----------------
            with (
                tc.tile_pool(name="xa", bufs=1) as xa,
                tc.tile_pool(name="wload", bufs=3) as wl,
                tc.tile_pool(name="psA", bufs=6, space="PSUM") as psA,
            ):
                xt = xa.tile([128, 8, SP], F32R, tag="xt")
                xtl = xa.tile([1, SP], F32R, tag="xtl")
                nc.sync.dma_start(
                    out=xt,
                    in_=xt_d[0:D, :].rearrange("(c p) j -> p c j", p=128),
                )
                nc.sync.dma_start(out=xtl, in_=xt_d[D : D + 1, :])

                # Q (channels 0:1024) and K (1024:2048), transposed layout
                for ct in range(16):
                    w = wl.tile([128, 8, 128], F32R, tag="wqk")
                    nc.sync.dma_start(
                        out=w,
                        in_=wqk_d[0:D, ct * 128 : (ct + 1) * 128].rearrange(
                            "(c p) m -> p c m", p=128
                        ),
                    )
                    blocks = (
                        [(0, 512), (512, 512), (1023, 2)] if ct < 8 else KB
                    )
                    for i0, iw in blocks:
                        pa = psA.tile([128, 512], F32, tag="psA")
                        for ec in range(8):
                            nc.tensor.matmul(
                                pa[:, :iw],
                                w[:, ec, :],
                                xt[:, ec, i0 : i0 + iw],
                                start=(ec == 0),
                                stop=False,
                            )
                        nc.tensor.matmul(
                            pa[:, :iw],
                            wql[0:1, ct * 128 : (ct + 1) * 128],
                            xtl[0:1, i0 : i0 + iw],
                            start=False,
                            stop=True,
                        )
                        dst = qt if ct < 8 else kt
                        nc.vector.tensor_copy(
                            dst[:, ct % 8, i0 : i0 + iw], pa[:, :iw]
                        )

                # V, natural layout, with bias via ones-row K=1
                for cbi, (c0, cw) in enumerate(CBV):
                    wv = wl.tile([128, 8, 384], F32R, tag="wv")
                    nc.sync.dma_start(
                        out=wv[:, :, :cw],
                        in_=wv_d[0:D, c0 : c0 + cw].rearrange(
                            "(c p) m -> p c m", p=128
                        ),
                    )
                    for jt in range(NJT):
                        pa = psA.tile([128, 512], F32, tag="psA")
                        for ec in range(8):
                            nc.tensor.matmul(
                                pa[:, :cw],
                                xt[:, ec, jt * 128 : (jt + 1) * 128],
                                wv[:, ec, :cw],
                                start=(ec == 0),
                                stop=False,
                            )
                        nc.tensor.matmul(
                            pa[:, :cw],
                            xtl[0:1, jt * 128 : (jt + 1) * 128],
                            wvl[0:1, c0 : c0 + cw],
                            start=False,
                            stop=True,
                        )
                        h0 = c0 // 64
                        nh = cw // 64
                        nc.vector.tensor_copy(
                            va[:, jt, h0 : h0 + nh, 0:64],
                            pa[:, :cw].rearrange("p (h c) -> p h c", c=64),
                        )

            # ---------------- Phase B: attention + proj ----------------
            with (
                tc.tile_pool(name="biasp", bufs=2) as bp,
                tc.tile_pool(name="attnp", bufs=2) as ap,
                tc.tile_pool(name="normp", bufs=1) as rp,
                tc.tile_pool(name="outp", bufs=1) as op,
                tc.tile_pool(name="projw", bufs=2) as pw,
                tc.tile_pool(name="yp", bufs=1) as yp,
                tc.tile_pool(name="dramp", bufs=2, space="DRAM") as dp,
                tc.tile_pool(name="psS", bufs=1, space="PSUM") as psS,
                tc.tile_pool(name="psPV", bufs=1, space="PSUM") as psPV,
            ):
                # width-1 column (i=1024), emitted interleaved with ib=0 pairs
                def emit_width1(p):
                    h0, h1 = 2 * p, 2 * p + 1
                    sx = psS.tile([128, 3, 512], F32, tag="s0")
                    sx1 = psS.tile([128, 3, 512], F32, tag="s1")
                    for hh, (sps, hid) in enumerate(((sx, h0), (sx1, h1))):
                        sp_t, hcur = sps, hid
                        base = (hcur % 2) * 64
                        for jt in range(NJT):
                            nc.tensor.matmul(
                                sp_t[:, 0, 2 * jt : 2 * jt + 2],
                                kt[base : base + 64, p, jt * 128 : (jt + 1) * 128],
                                qt[base : base + 64, p, 1023:1025],
                                start=True,
                                stop=True,
                                skip_group_check=True,
                            )
                    b1 = rp.tile([128, 2, NJT, 2], BF16, tag="b1w")
                    for hh, hcur in enumerate((h0, h1)):
                        nc.sync.dma_start(
                            out=b1[:, hh, :, :],
                            in_=bias1_d[hcur, :, :].rearrange("(t p) i -> p t i", p=128),
                        )
                    nc.vector.tensor_add(
                        sx[:, 0, 0 : 2 * NJT], sx[:, 0, 0 : 2 * NJT],
                        b1[:, 0, :, :].rearrange("p t i -> p (t i)"),
                    )
                    nc.vector.tensor_add(
                        sx1[:, 0, 0 : 2 * NJT], sx1[:, 0, 0 : 2 * NJT],
                        b1[:, 1, :, :].rearrange("p t i -> p (t i)"),
                    )
                    e1x = ap.tile([128, 2, 2 * NJT], F16, tag="e1x")
                    nc.scalar.activation(e1x[:, 0, :], sx[:, 0, 0 : 2 * NJT], EXPFN, scale=0.125)
                    nc.scalar.activation(e1x[:, 1, :], sx1[:, 0, 0 : 2 * NJT], EXPFN, scale=0.125)
                    pv0 = psPV.tile([128, 512], F32, tag="pv0")
                    pv1 = psPV.tile([128, 512], F32, tag="pv1")
                    for jt in range(NJT):
                        nc.tensor.matmul(
                            pv0[0:65, 0:1], va[:, jt, h0, :], e1x[:, 0, 2 * jt + 1 : 2 * jt + 2],
                            start=(jt == 0), stop=(jt == NJT - 1), skip_group_check=True,
                        )
                        nc.tensor.matmul(
                            pv1[0:65, 0:1], va[:, jt, h1, :], e1x[:, 1, 2 * jt + 1 : 2 * jt + 2],
                            start=(jt == 0), stop=(jt == NJT - 1), skip_group_check=True,
                        )
                    pvsb1 = rp.tile([128, 4], F32, tag="pvsb1")
                    nc.vector.tensor_copy(pvsb1[0:64, 0:1], pv0[0:64, 0:1])
                    nc.vector.tensor_copy(pvsb1[64:128, 0:1], pv1[0:64, 0:1])
                    rec = rp.tile([1, 2, 4], F32, tag="rec1")
                    nc.vector.reciprocal(rec[0:1, 0, 0:1], pv0[64:65, 0:1])
                    nc.vector.reciprocal(rec[0:1, 1, 0:1], pv1[64:65, 0:1])
                    rdr = dp.tile([2, 1], F32, tag="rdr1")
                    nc.sync.dma_start(out=rdr, in_=rec[0:1, :, 0:1])
                    bc = rp.tile([128, 4], F32, tag="bc1")
                    nc.gpsimd.dma_start(
                        out=bc[:, 0:1],
                        in_=bass.AP(tensor=rdr.tensor, offset=rdr.offset,
                                    ap=[[1, 2], [0, 64], [1, 1]]),
                    )
                    nc.vector.tensor_mul(out1[0:64, p, :], pvsb1[0:64, 0:1], bc[0:64, 0:1])
                    nc.vector.tensor_mul(out1[64:128, p, :], pvsb1[64:128, 0:1], bc[64:128, 0:1])

                # ---- main i-blocks ----
                for ib, (i0, iw) in enumerate(IB):
                    out_all = op.tile([128, 8, 512], F32R, tag="out_all")
                    for p in range(8):
                        h0, h1 = 2 * p, 2 * p + 1
                        pv0 = psPV.tile([128, 512], F32, tag="pv0")
                        pv1 = psPV.tile([128, 512], F32, tag="pv1")
                        pend = None
                        for g0, gn in ((0, 3), (3, 3), (6, 3), (9, 0)):
                            if gn > 0:
                                b0 = bp.tile([128, 3, 512], BF16, tag="b0")
                                b1 = bp.tile([128, 3, 512], BF16, tag="b1")
                                nc.sync.dma_start(
                                    out=b0[:, :gn, :],
                                    in_=bias_d[
                                        h0, g0 * 128 : (g0 + gn) * 128, i0 : i0 + iw
                                    ].rearrange("(t p) i -> p t i", p=128),
                                )
                                nc.sync.dma_start(
                                    out=b1[:, :gn, :],
                                    in_=bias_d[
                                        h1, g0 * 128 : (g0 + gn) * 128, i0 : i0 + iw
                                    ].rearrange("(t p) i -> p t i", p=128),
                                )
                                s0 = psS.tile([128, 3, 512], F32, tag="s0")
                                s1 = psS.tile([128, 3, 512], F32, tag="s1")
                                for seg in range(gn):
                                    jt = g0 + seg
                                    js = slice(jt * 128, (jt + 1) * 128)
                                    nc.tensor.matmul(
                                        s0[:, seg, :], kt[0:64, p, js], qt[0:64, p, i0 : i0 + iw],
                                        start=True, stop=False, skip_group_check=True,
                                    )
                                    nc.tensor.matmul(
                                        s1[:, seg, :], kt[64:128, p, js], qt[64:128, p, i0 : i0 + iw],
                                        start=True, stop=False, skip_group_check=True,
                                    )
                                    nc.tensor.matmul(
                                        s0[:, seg, :], iden, b0[:, seg, :],
                                        start=False, stop=True, skip_group_check=True,
                                    )
                                    nc.tensor.matmul(
                                        s1[:, seg, :], iden, b1[:, seg, :],
                                        start=False, stop=True, skip_group_check=True,
                                    )
                                e0 = ap.tile([128, 3, 512], F16, tag="e0")
                                e1 = ap.tile([128, 3, 512], F16, tag="e1")
                                nc.scalar.activation(e0[:, :gn, :], s0[:, :gn, :], EXPFN, scale=0.125)
                                nc.scalar.activation(e1[:, :gn, :], s1[:, :gn, :], EXPFN, scale=0.125)
                            # PV for the previous group (software pipelined)
                            if pend is not None:
                                pe0, pe1, pg0, pgn = pend
                                for seg in range(pgn):
                                    jt = pg0 + seg
                                    nc.tensor.matmul(
                                        pv0[0:65, :], va[:, jt, h0, :], pe0[:, seg, :],
                                        start=(jt == 0), stop=(jt == NJT - 1),
                                        skip_group_check=True,
                                    )
                                    nc.tensor.matmul(
                                        pv1[0:65, :], va[:, jt, h1, :], pe1[:, seg, :],
                                        start=(jt == 0), stop=(jt == NJT - 1),
                                        skip_group_check=True,
                                    )
                            if gn > 0:
                                pend = (e0, e1, g0, gn)
                        pvsb = rp.tile([128, 512], F32, tag="pvsb")
                        nc.vector.tensor_copy(pvsb[0:64, :], pv0[0:64, :])
                        nc.vector.tensor_copy(pvsb[64:128, :], pv1[0:64, :])
                        rec = rp.tile([1, 2, 512], F32, tag="rec")
                        nc.vector.reciprocal(rec[0:1, 0, :], pv0[64:65, :])
                        nc.vector.reciprocal(rec[0:1, 1, :], pv1[64:65, :])
                        rdr = dp.tile([2, 512], F32, tag="rdr")
                        nc.sync.dma_start(out=rdr, in_=rec[0:1, :, :])
                        bc = rp.tile([128, 512], F32, tag="bc")
                        nc.gpsimd.dma_start(
                            out=bc,
                            in_=bass.AP(tensor=rdr.tensor, offset=rdr.offset,
                                        ap=[[512, 2], [0, 64], [1, 512]]),
                        )
                        nc.vector.tensor_mul(out_all[0:64, p, :], pvsb[0:64, :], bc[0:64, :])
                        nc.vector.tensor_mul(out_all[64:128, p, :], pvsb[64:128, :], bc[64:128, :])
                        if ib == 0:
                            emit_width1(p)

                    # proj for this i-block
                    ysb = yp.tile([128, 4, D], F32, tag="ysb")
                    y1 = yp.tile([1, D], F32, tag="y1")
                    wpjs = []
                    for f0, fw in FB:
                        wpj = pw.tile([128, 8, 512], F32R, tag="wpj")
                        nc.sync.dma_start(
                            out=wpj[:, :, :fw],
                            in_=wp_d[0:D, f0 : f0 + fw].rearrange(
                                "(c p) m -> p c m", p=128
                            ),
                        )
                        wpjs.append(wpj)
                    for (f0, fw), wpj in zip(FB, wpjs):
                        for it in range(4):
                            pj = psPV.tile([128, 512], F32, tag="pv0")
                            for cc in range(8):
                                nc.tensor.matmul(
                                    pj[:, :fw],
                                    out_all[:, cc, it * 128 : (it + 1) * 128],
                                    wpj[:, cc, :fw],
                                    start=(cc == 0), stop=False,
                                    skip_group_check=True,
                                )
                            nc.tensor.matmul(
                                pj[:, :fw], ones_row[0:1, 0:128], wpl[0:1, f0 : f0 + fw],
                                start=False, stop=True, skip_group_check=True,
                            )
                            nc.vector.tensor_copy(ysb[:, it, f0 : f0 + fw], pj[:, :fw])
                        if ib == 1:
                            # the single i=1024 row rides the second block's weights
                            pj1 = psPV.tile([128, 512], F32, tag="pv1")
                            for cc in range(8):
                                nc.tensor.matmul(
                                    pj1[0:1, :fw], out1[:, cc, 0:1], wpj[:, cc, :fw],
                                    start=(cc == 0), stop=False, skip_group_check=True,
                                )
                            nc.tensor.matmul(
                                pj1[0:1, :fw], ones_row[0:1, 0:1], wpl[0:1, f0 : f0 + fw],
                                start=False, stop=True, skip_group_check=True,
                            )
                            nc.vector.tensor_copy(y1[0:1, f0 : f0 + fw], pj1[0:1, :fw])
                    nc.sync.dma_start(
                        out=y_d[i0 : i0 + iw, :].rearrange("(t p) f -> p t f", p=128),
                        in_=ysb,
                    )
                    if ib == 1:
                        nc.sync.dma_start(out=y_d[1024:1025, :], in_=y1)

    nc.finalize()
    return nc


def _prepare_inputs(x, qkv_w, qkv_b, proj_w, proj_b, rel_pos_table, rel_pos_idx):
    """Host-side sharding/layout prep. Returns per-core input maps."""
    xf = np.asarray(x, dtype=np.float32)
    wqkv_aug = np.empty((SEQ, 3 * D), np.float32)
    wqkv_aug[0:D] = np.asarray(qkv_w, np.float32).T
    wqkv_aug[D] = np.asarray(qkv_b, np.float32)
    wqkv_aug = _f32r_round(wqkv_aug)
    wqk = np.ascontiguousarray(wqkv_aug[:, 0 : 2 * D])
    wv = np.ascontiguousarray(wqkv_aug[:, 2 * D : 3 * D])

    wp = np.empty((SEQ, D), np.float32)
    wp[0:D] = np.asarray(proj_w, np.float32).T
    wp[D] = np.asarray(proj_b, np.float32)
    wp = _f32r_round(wp)

    table8 = 8.0 * np.asarray(rel_pos_table, np.float32)        # [ndist, H]
    idx = np.asarray(rel_pos_idx)
    g = table8[idx]                                             # [i, j, H]
    biasT = np.full((H, SP, SEQ), NEG, np.float32)
    biasT[:, 0:SEQ, :] = g.transpose(2, 1, 0)                   # [H, j, i]
    biasT = biasT.astype(ml_dtypes.bfloat16)
    bias1 = np.ascontiguousarray(biasT[:, :, SEQ - 2 : SEQ])    # [H, SP, 2]

    iden = np.eye(128, dtype=np.float32).astype(ml_dtypes.bfloat16)

    in_maps = []
    for b in range(NB):
        xt = np.zeros((SEQ, SP), np.float32)
        xt[0:D, 0:SEQ] = xf[b].T
        xt[D, 0:SEQ] = 1.0
        xt = _f32r_round(xt)
        in_maps.append(
            {
                "xt": xt, "wqk": wqk, "wv": wv, "wp": wp,
                "biasT": biasT, "bias1": bias1, "iden": iden,
            }
        )
    return in_maps


def run(inputs, trace=False):
    """Compile (cached) + run on 8 cores. Returns (out [8,1025,1024], results)."""
    if "nc" not in _CACHE:
        _CACHE["nc"] = _build_module()
    nc = _CACHE["nc"]
    in_maps = _prepare_inputs(**inputs)
    res = run_bass_kernel_spmd(
        nc, in_maps, core_ids=list(range(NB)), trace=trace,
        trace_cores=[0] if trace else None,
    )
    out = np.stack([res.results[b]["y"] for b in range(NB)], axis=0)
    return out, res


def kernel(**inputs) -> np.ndarray:
    out, _ = run(inputs, trace=False)
    return out



# revision 5
# speedup vs baseline: 1.1145x; 1.1145x over previous
"""Trainium2 Bass kernel: BEiT-style dot-product attention with relative
position bias (batch 8, seq 1025, dim 1024, 16 heads), data-parallel over
batch (one batch element per NeuronCore).

v2 design:
  - All GEMM operands bf16 (FWL weight loads, no fp32r self-load stalls).
  - Multiplicative bias: exp(s+b) = exp(s) * exp(b). ScalarE computes
    exp(s*0.125) straight from the QK psum; the host-precomputed f16
    exp(bias) factor is applied by a 2x-rate DVE multiply. No bias matmuls,
    no f32 bias adds, pad key j=1025 masked by exp(bias)=0.
  - seq padded 1025 -> 1026: 9 j-tiles x 114 rows, 3 i-blocks x 342 cols
    (no special width-1 column path).
  - Scores: row-packed QK pairs (2 heads: K=64 rows 0-63 / 64-127).
  - PV: per head stationary [114, 65] = [v | ones]; psum row 64 gives the
    softmax denominator for free.
  - Normalization: reciprocal_approx_fast on the two denominator rows,
    DRAM-roundtrip broadcast to 128 partitions, fused psum-drain multiply.
  - Software pipelining: PV of pair p-1 interleaved with QK of pair p.
  - qkv/proj biases folded into psum drains (per-partition scalar for Q/K,
    broadcast row tiles for V/proj).
"""

import os
import sys

for _p in (
    "/root/.axon_site",
    "/root/.axon_site/_ro/trn_rl_repo",
    "/root/.axon_site/_ro/pypackages",
    "/opt/trn_rl_repo",
    "/opt/pypackages",
):
    if os.path.isdir(_p) and _p not in sys.path:
        sys.path.append(_p)

import numpy as np
import ml_dtypes

import concourse.bass as bass
import concourse.bacc as bacc
import concourse.tile as tile
import concourse.mybir as mybir
from concourse.bass_utils import run_bass_kernel_spmd

F32 = mybir.dt.float32
BF16 = mybir.dt.bfloat16
F16 = mybir.dt.float16
EXPFN = mybir.ActivationFunctionType.Exp

SEQ = 1025
SP = 1026            # padded seq (9 j-tiles of 114; i padded with one zero col)
JT = 114             # j-tile rows
NJT = 9
D = 1024
H = 16
NB = 8
IBW = 342            # i-block width (3 blocks of 342 = 1026)
NIB = 3
QBLK = [(0, 512), (512, 512), (1024, 2)]

_CACHE = {}


def _build_module():
    nc = bacc.Bacc()
    xt_d = nc.dram_tensor("xt", [128, 8, SP], BF16, kind="ExternalInput")
    wq_d = nc.dram_tensor("wq", [8, 128, 8, 128], BF16, kind="ExternalInput")
    wk_d = nc.dram_tensor("wk", [8, 128, 8, 128], BF16, kind="ExternalInput")
    wv_d = nc.dram_tensor("wv", [128, 8, D], BF16, kind="ExternalInput")
    wp_d = nc.dram_tensor("wp", [128, 8, D], BF16, kind="ExternalInput")
    qb_d = nc.dram_tensor("qb", [128, 8], F32, kind="ExternalInput")
    kb_d = nc.dram_tensor("kb", [128, 8], F32, kind="ExternalInput")
    vb_d = nc.dram_tensor("vb", [1, D], BF16, kind="ExternalInput")
    pb_d = nc.dram_tensor("pb", [1, D], BF16, kind="ExternalInput")
    # exp(bias): [ib, head, p(114), jt(9), i(342)] f16, contiguous per (ib, head)
    eb_d = nc.dram_tensor("eb", [NIB, H, JT, NJT, IBW], F16, kind="ExternalInput")
    y_d = nc.dram_tensor("y", [SEQ, D], F32, kind="ExternalOutput")

    with tile.TileContext(nc) as tc:
        with (
            tc.tile_pool(name="persist", bufs=1) as pp,
            tc.tile_pool(name="consts", bufs=1) as cp,
        ):
            qt = pp.tile([128, 8, SP], BF16, tag="qt")
            kt = pp.tile([128, 8, SP], BF16, tag="kt")
            va = pp.tile([128, NJT, H, 65], F16, tag="va")
            out_all = pp.tile([128, 8, SP], BF16, tag="out_all")

            qb = cp.tile([128, 8], F32, tag="qb")
            kb = cp.tile([128, 8], F32, tag="kb")
            vb = cp.tile([128, D], BF16, tag="vb")
            pb = cp.tile([128, D], BF16, tag="pb")
            nc.sync.dma_start(out=qb, in_=qb_d[:, :])
            nc.sync.dma_start(out=kb, in_=kb_d[:, :])
            nc.gpsimd.dma_start(
                out=vb,
                in_=bass.AP(tensor=vb_d, offset=0, ap=[[0, 128], [1, D]]),
            )
            nc.gpsimd.dma_start(
                out=pb,
                in_=bass.AP(tensor=pb_d, offset=0, ap=[[0, 128], [1, D]]),
            )
            onesf = cp.tile([128, NJT * H], F32, tag="onesf")
            nc.vector.memset(onesf, 1.0)
            nc.vector.tensor_copy(
                va[:, :, :, 64:65],
                onesf.rearrange("p (t h) -> p t h", t=NJT).unsqueeze(3),
            )

            # ---------------- Phase A: Q, K, V projections ----------------
            with (
                tc.tile_pool(name="xa", bufs=1) as xa,
                tc.tile_pool(name="wl", bufs=3) as wl,
                tc.tile_pool(name="psA", bufs=4, space="PSUM") as psA,
            ):
                xt = xa.tile([128, 8, SP], BF16, tag="xt")
                nc.sync.dma_start(out=xt, in_=xt_d[:, :, :])

                for dst, wsrc, bias in ((qt, wq_d, qb), (kt, wk_d, kb)):
                    for ct in range(8):
                        w = wl.tile([128, 8, 128], BF16, tag="wqk")
                        nc.sync.dma_start(out=w, in_=wsrc[ct])
                        for i0, iw in QBLK:
                            pa = psA.tile([128, 512], F32, tag="psA")
                            for ec in range(8):
                                nc.tensor.matmul(
                                    pa[:, :iw],
                                    w[:, ec, :],
                                    xt[:, ec, i0 : i0 + iw],
                                    start=(ec == 0),
                                    stop=(ec == 7),
                                    skip_group_check=True,
                                )
                            nc.vector.tensor_scalar_add(
                                dst[:, ct, i0 : i0 + iw],
                                pa[:, :iw],
                                bias[:, ct : ct + 1],
                            )

                wv = xa.tile([128, 8, D], BF16, tag="wv")
                nc.sync.dma_start(out=wv, in_=wv_d[:, :, :])
                for jt in range(NJT):
                    js = slice(jt * JT, (jt + 1) * JT)
                    for hb in range(2):
                        ms = slice(hb * 512, (hb + 1) * 512)
                        pa = psA.tile([128, 512], F32, tag="psA")
                        for ec in range(8):
                            nc.tensor.matmul(
                                pa[:JT, :],
                                xt[:, ec, js],
                                wv[:, ec, ms],
                                start=(ec == 0),
                                stop=(ec == 7),
                                skip_group_check=True,
                            )
                        nc.vector.tensor_add(
                            va[:JT, jt, hb * 8 : (hb + 1) * 8, 0:64],
                            pa[:JT, :].rearrange("p (h c) -> p h c", c=64),
                            vb[:JT, ms].rearrange("p (h c) -> p h c", c=64),
                        )

            # ---------------- Phase B: attention ----------------
            with (
                tc.tile_pool(name="psS", bufs=3, space="PSUM") as psS,
                tc.tile_pool(name="psPV", bufs=1, space="PSUM") as psPV,
                tc.tile_pool(name="ebp", bufs=2) as ebp,
                tc.tile_pool(name="exps", bufs=2) as xp,
                tc.tile_pool(name="ep", bufs=2) as ep,
                tc.tile_pool(name="rp", bufs=2) as rp,
                tc.tile_pool(name="dramp", bufs=2, space="DRAM") as dp,
            ):
                def qk_score(p, ib, jt, pvt_prev, e_prev, p_prev):
                    """Emit QK pair for (p, jt) interleaved with PV of prev pair."""
                    i0 = ib * IBW
                    js = slice(jt * JT, (jt + 1) * JT)
                    s = psS.tile([128, 2, 512], F32, tag="s2")
                    nc.tensor.matmul(
                        s[:JT, 0, :IBW],
                        kt[0:64, p, js],
                        qt[0:64, p, i0 : i0 + IBW],
                        start=True, stop=True,
                        skip_group_check=True,
                        tile_position=(0, 0),
                    )
                    nc.tensor.matmul(
                        s[:JT, 1, :IBW],
                        kt[64:128, p, js],
                        qt[64:128, p, i0 : i0 + IBW],
                        start=True, stop=True,
                        skip_group_check=True,
                        tile_position=(64, 0),
                    )
                    if pvt_prev is not None:
                        pv_acc(p_prev, jt, pvt_prev, e_prev)
                    return s

                def pv_acc(p, jt, pvt, e):
                    for hh in range(2):
                        nc.tensor.matmul(
                            pvt[0:65, hh, :IBW],
                            va[:JT, jt, 2 * p + hh, :],
                            e[:JT, jt, hh, :],
                            start=(jt == 0),
                            stop=(jt == NJT - 1),
                            skip_group_check=True,
                        )

                def finish_pair(p, ib, pvt):
                    """Normalize + drain pair p's PV accumulators."""
                    i0 = ib * IBW
                    dn = rp.tile([1, 2, IBW], F32, tag="dn")
                    nc.vector.tensor_copy(dn, pvt[64:65, :, :IBW])
                    rr = rp.tile([1, 2, IBW], F32, tag="rr")
                    nc.vector.reciprocal_approx_fast(out=rr, in_=dn)
                    rd = dp.tile([2, IBW], F32, tag="rd")
                    nc.sync.dma_start(out=rd, in_=rr[0:1, :, :])
                    bc = rp.tile([128, IBW], F32, tag="bc")
                    nc.gpsimd.dma_start(
                        out=bc,
                        in_=bass.AP(
                            tensor=rd.tensor, offset=rd.offset,
                            ap=[[IBW, 2], [0, 64], [1, IBW]],
                        ),
                    )
                    tmp = rp.tile([128, IBW], F32, tag="tmp")
                    nc.vector.tensor_mul(
                        out_all[0:64, p, i0 : i0 + IBW],
                        pvt[0:64, 0, :IBW],
                        bc[0:64, :],
                    )
                    nc.vector.tensor_copy(tmp[64:128, :], pvt[0:64, 1, :IBW])
                    nc.vector.tensor_mul(
                        out_all[64:128, p, i0 : i0 + IBW],
                        tmp[64:128, :],
                        bc[64:128, :],
                    )

                prev = None  # (p, pvt, e)
                for ib in range(NIB):
                    for p in range(8):
                        eb0 = ebp.tile([128, NJT, IBW], F16, tag="eb0")
                        eb1 = ebp.tile([128, NJT, IBW], F16, tag="eb1")
                        nc.sync.dma_start(out=eb0[:JT], in_=eb_d[ib, 2 * p])
                        nc.sync.dma_start(out=eb1[:JT], in_=eb_d[ib, 2 * p + 1])
                        exps = xp.tile([128, NJT, 2, IBW], F16, tag="exps")
                        for jt in range(NJT):
                            if prev is not None:
                                s = qk_score(p, ib, jt, prev[1], prev[2], prev[0] % 8)
                            else:
                                s = qk_score(p, ib, jt, None, None, None)
                            nc.scalar.activation(
                                exps[:JT, jt, :, :],
                                s[:JT, :, :IBW],
                                EXPFN,
                                scale=0.125,
                            )
                        if prev is not None:
                            finish_pair(prev[0] % 8, prev[0] // 8, prev[1])
                        e = ep.tile([128, NJT, 2, IBW], F16, tag="e")
                        nc.vector.tensor_mul(
                            e[:JT, :, 0, :], exps[:JT, :, 0, :], eb0[:JT, :, :]
                        )
                        nc.vector.tensor_mul(
                            e[:JT, :, 1, :], exps[:JT, :, 1, :], eb1[:JT, :, :]
                        )
                        pvt = psPV.tile([128, 2, 512], F32, tag="pvt")
                        prev = (ib * 8 + p, pvt, e)
                # flush last pair
                for jt in range(NJT):
                    pv_acc(prev[0] % 8, jt, prev[1], prev[2])
                finish_pair(prev[0] % 8, prev[0] // 8, prev[1])

            # ---------------- Phase C: output projection ----------------
            with (
                tc.tile_pool(name="wpp", bufs=1) as wpp,
                tc.tile_pool(name="yp", bufs=2) as yp,
                tc.tile_pool(name="psJ", bufs=2, space="PSUM") as psJ,
            ):
                wp = wpp.tile([128, 8, D], BF16, tag="wp")
                nc.sync.dma_start(out=wp, in_=wp_d[:, :, :])
                for mt in range(9):
                    i0 = 128 * mt if mt < 8 else SEQ - 128
                    ysb = yp.tile([128, D], F32, tag="ysb")
                    for fb in range(2):
                        fs = slice(fb * 512, (fb + 1) * 512)
                        pj = psJ.tile([128, 512], F32, tag="pj")
                        for cc in range(8):
                            nc.tensor.matmul(
                                pj,
                                out_all[:, cc, i0 : i0 + 128],
                                wp[:, cc, fs],
                                start=(cc == 0),
                                stop=(cc == 7),
                                skip_group_check=True,
                            )
                        nc.vector.tensor_add(ysb[:, fs], pj, pb[:, fs])
                    if mt < 8:
                        nc.sync.dma_start(out=y_d[i0 : i0 + 128, :], in_=ysb)
                    else:
                        nc.sync.dma_start(
                            out=y_d[SEQ - 1 : SEQ, :], in_=ysb[127:128, :]
                        )

    nc.finalize()
    return nc


def _prepare_inputs(x, qkv_w, qkv_b, proj_w, proj_b, rel_pos_table, rel_pos_idx):
    bf = ml_dtypes.bfloat16
    xf = np.asarray(x, np.float32)
    qkv_w = np.asarray(qkv_w, np.float32)
    qkv_b = np.asarray(qkv_b, np.float32)
    proj_w = np.asarray(proj_w, np.float32)
    proj_b = np.asarray(proj_b, np.float32)

    wq = np.ascontiguousarray(
        qkv_w[0:D].reshape(8, 128, 8, 128).transpose(0, 3, 2, 1)
    ).astype(bf)
    wk = np.ascontiguousarray(
        qkv_w[D : 2 * D].reshape(8, 128, 8, 128).transpose(0, 3, 2, 1)
    ).astype(bf)
    wv = np.ascontiguousarray(
        qkv_w[2 * D : 3 * D].reshape(D, 8, 128).transpose(2, 1, 0)
    ).astype(bf)
    wp = np.ascontiguousarray(
        proj_w.reshape(D, 8, 128).transpose(2, 1, 0)
    ).astype(bf)
    qb = np.ascontiguousarray(qkv_b[0:D].reshape(8, 128).T).astype(np.float32)
    kb = np.ascontiguousarray(qkv_b[D : 2 * D].reshape(8, 128).T).astype(np.float32)
    vb = qkv_b[2 * D : 3 * D].reshape(1, D).astype(bf)
    pbr = proj_b.reshape(1, D).astype(bf)

    # exp(bias) factor [ib, h, p, jt, i] f16; pad key j=1025 -> 0
    g = np.asarray(rel_pos_table, np.float32)[np.asarray(rel_pos_idx)]  # [i,j,H]
    full = np.zeros((H, SP, SP), np.float32)                            # [h,j,i]
    full[:, :SEQ, :SEQ] = np.exp(g).transpose(2, 1, 0)
    full[:, :, SEQ:] = 1.0
    full[:, SEQ:, :] = 0.0
    eb = np.ascontiguousarray(
        full.reshape(H, NJT, JT, NIB, IBW).transpose(3, 0, 2, 1, 4)
    ).astype(np.float16)

    in_maps = []
    for b in range(NB):
        xt = np.zeros((128, 8, SP), np.float32)
        xt[:, :, :SEQ] = xf[b].T.reshape(8, 128, SEQ).transpose(1, 0, 2)
        xt = xt.astype(bf)
        in_maps.append(
            {
                "xt": xt, "wq": wq, "wk": wk, "wv": wv, "wp": wp,
                "qb": qb, "kb": kb, "vb": vb, "pb": pbr, "eb": eb,
            }
        )
    return in_maps


def run(inputs, trace=False):
    if "nc" not in _CACHE:
        _CACHE["nc"] = _build_module()
    nc = _CACHE["nc"]
    in_maps = _prepare_inputs(**inputs)
    res = run_bass_kernel_spmd(
        nc, in_maps, core_ids=list(range(NB)), trace=trace,
        trace_cores=[0] if trace else None,
    )
    out = np.stack([res.results[b]["y"] for b in range(NB)], axis=0)
    return out, res


def kernel(**inputs) -> np.ndarray:
    out, _ = run(inputs, trace=False)
    return out
